# revision 1
# baseline (speedup 1.0000x reference)
"""Trainium2 Bass kernel for nn_CNN_80221399155117.

Pipeline: full-vocab softmax -> token-prob gather -> -log2 surprisal ->
concat(hidden, surp) -> Conv1d(k=5, pad=2) -> MaxPool1d(5) -> ReLU -> FC.

Sharding: 8 cores = (batch b, seq-half h). Each core owns the pool-aligned
conv-output range [510h, 510h+510) of its batch, needing feats rows
[510h-2, 510h+512) (EXT=514, zero-padded outside [0,1024)). The softmax
normalizer is computed locally per row (positions sharded, vocab local),
so no collectives are needed. The token-logit gather runs on-device via
indirect DMA with flat indices built from iota + input_ids.
"""

import numpy as np

B, S, V, H = 4, 1024, 32000, 2048
OC, K = 128, 5
N_CORES = 8
Y_LOC = 510            # conv output positions per core (102 pool windows)
PO_LOC = 102           # pooled cols per core
EXT = 514              # feats rows incl conv halo (510 + 2 + 2)
CF = 4000              # vocab chunk (free-dim) size
NCH = V // CF          # 8 chunks
LOG2E = 1.4426950408889634

_CACHE = {}
VARIANT = "indirect"   # bisect knob: indirect | nogather | flat2d | nopass1 | noconv


def _build_program():
    import concourse.tile as tile
    from concourse import bacc, bass, mybir
    from concourse.masks import make_identity

    f32 = mybir.dt.float32
    i32 = mybir.dt.int32
    Alu = mybir.AluOpType
    Act = mybir.ActivationFunctionType

    nc = bacc.Bacc("TRN2", target_bir_lowering=False, debug=False,
                   num_devices=N_CORES)

    logits = nc.dram_tensor("logits_loc", [EXT, V], f32, kind="ExternalInput").ap()
    ids = nc.dram_tensor("ids_loc", [EXT, 1], i32, kind="ExternalInput").ap()
    maskd = nc.dram_tensor("mask_loc", [EXT, 1], f32, kind="ExternalInput").ap()
    hid = nc.dram_tensor("hidden_loc", [EXT, H], f32, kind="ExternalInput").ap()
    wt = nc.dram_tensor("wt", [H, K * OC], f32, kind="ExternalInput").ap()
    wsurp = nc.dram_tensor("wsurp", [K, OC], f32, kind="ExternalInput").ap()
    convb = nc.dram_tensor("convb", [OC, 1], f32, kind="ExternalInput").ap()
    fcw = nc.dram_tensor("fcw", [OC, 3 * PO_LOC], f32, kind="ExternalInput").ap()
    sentv = nc.dram_tensor("sentv", [128, 1], f32, kind="ExternalInput").ap()
    sentw = nc.dram_tensor("sentw", [128, 3], f32, kind="ExternalInput").ap()
    fcb = nc.dram_tensor("fcb", [3, 1], f32, kind="ExternalInput").ap()
    out = nc.dram_tensor("out_loc", [3, 1], f32, kind="ExternalOutput").ap()

    surp_dram = nc.dram_tensor("surp_scratch", [1, EXT], f32).ap()

    logits_flat = bass.AP(logits.tensor, 0, [[1, EXT * V], [1, 1]])

    ROW_TILES = [(0, 128), (128, 128), (256, 128), (384, 128)]
    NHALO = EXT - 512                  # 2 halo rows, packed [128, HF]
    HQ = 128 // NHALO                  # partitions per halo row
    HF = V // HQ                       # free elems per partition

    with tile.TileContext(nc) as tc:
        with (
            tc.tile_pool(name="lp", bufs=6) as lp,          # logits chunks
            tc.tile_pool(name="scr", bufs=2) as scr,        # exp scratch
            tc.tile_pool(name="big", bufs=1) as big,        # resident X / weights
            tc.tile_pool(name="hn", bufs=2) as hnp,         # hidden natural tiles
            tc.tile_pool(name="sm", bufs=12) as sm,         # small per-tile stats
            tc.tile_pool(name="ps_t", bufs=4, space="PSUM") as ps_t,
            tc.tile_pool(name="ps_y", bufs=1, space="PSUM") as ps_y,
            tc.tile_pool(name="ps_o", bufs=1, space="PSUM") as ps_o,
        ):
            # ---- resident constants ----
            ident = big.tile([128, 128], f32, tag="ident")
            make_identity(nc, ident[:])
            f16 = mybir.dt.float16
            wtile = big.tile([128, 16 * K * OC], f16, tag="wtile")  # 16 ch-chunks
            for cc in range(16):
                nc.gpsimd.dma_start(        # SWDGE casts f32->bf16 in flight
                    out=wtile[:, cc * 640:(cc + 1) * 640],
                    in_=wt[cc * 128:(cc + 1) * 128, :],
                )
            wsurp_sb = big.tile([K, OC], f32, tag="wsurp")
            nc.sync.dma_start(out=wsurp_sb[:], in_=wsurp)
            convb_sb = big.tile([OC, 1], f32, tag="convb")
            nc.sync.dma_start(out=convb_sb[:], in_=convb)
            fcw_sb = big.tile([OC, 3 * PO_LOC], f32, tag="fcw")
            nc.sync.dma_start(out=fcw_sb[:], in_=fcw)
            sentv_sb = big.tile([128, 1], f32, tag="sentv")
            nc.sync.dma_start(out=sentv_sb[:], in_=sentv)
            sentw_sb = big.tile([128, 3], f32, tag="sentw")
            nc.sync.dma_start(out=sentw_sb[:], in_=sentw)
            fcb_sb = big.tile([3, 1], f32, tag="fcb")
            nc.sync.dma_start(out=fcb_sb[:], in_=fcb)
            ones_sb = big.tile([128, 1], f32, tag="ones")
            nc.vector.memset(ones_sb[:], 1.0)

            # ---- hidden -> transposed X tiles [ch, pos] ----
            xt = big.tile([128, 16 * EXT], f16, tag="xt")
            for r0, pn in ROW_TILES + [(512, NHALO)]:
                hn = hnp.tile([128, H], f32, tag="hn")
                nc.sync.dma_start(out=hn[:pn, :], in_=hid[r0:r0 + pn, :])
                for cc in range(16):
                    tp = ps_t.tile([128, 128], f32, tag="tp")
                    nc.tensor.transpose(
                        out=tp[:, :pn],
                        in_=hn[:pn, cc * 128:(cc + 1) * 128],
                        identity=ident[:pn, :pn],
                    )
                    nc.vector.tensor_copy(
                        out=xt[:, cc * EXT + r0: cc * EXT + r0 + pn],
                        in_=tp[:, :pn],
                    )

            # ---- conv: 80 hidden matmuls accumulate into one PSUM bank ----
            psum_y = ps_y.tile([OC, Y_LOC], f32, tag="y")
            first = True
            for cc in range(16):
                for k in range(K):
                    nc.tensor.matmul(
                        out=psum_y[:],
                        lhsT=wtile[:, cc * 640 + k * 128: cc * 640 + (k + 1) * 128],
                        rhs=xt[:, cc * EXT + k: cc * EXT + k + Y_LOC],
                        start=first,
                        stop=False,
                    )
                    first = False

            # ---- pass-1 shared stats, gathered upfront ----
            # cols 0..3 = main row tiles (row = 128*t + p), col 4 = halo rows
            NT = len(ROW_TILES)
            se_all = big.tile([128, NT + 1], f32, tag="se")    # sum(exp)
            g_all = big.tile([128, NT + 1], f32, tag="g")      # gathered logit
            m_all = big.tile([128, NT + 1], f32, tag="m")      # attention mask
            nc.vector.memset(se_all[:, NT:], 1.0)   # ln(1)=0 on unused lanes
            nc.vector.memset(g_all[:, NT:], 0.0)
            nc.vector.memset(m_all[:, NT:], 0.0)

            ids_all = sm.tile([128, NT], i32, tag="ids")
            nc.gpsimd.dma_start(out=ids_all[:],
                                in_=bass.AP(ids.tensor, 0, [[1, 128], [128, NT]]))
            nc.gpsimd.dma_start(out=m_all[:, :NT],
                                in_=bass.AP(maskd.tensor, 0, [[1, 128], [128, NT]]))
            nc.gpsimd.dma_start(out=m_all[:NHALO, NT:], in_=maskd[512:EXT, :])
            iota_t = sm.tile([128, NT], i32, tag="iota")
            nc.gpsimd.iota(iota_t[:], pattern=[[1, NT]], base=0,
                           channel_multiplier=0)
            nc.vector.tensor_scalar(out=iota_t[:], in0=iota_t[:],
                                    scalar1=128 * V, scalar2=None, op0=Alu.mult)
            iota_p = sm.tile([128, 1], i32, tag="iotap")
            nc.gpsimd.iota(iota_p[:], pattern=[[1, 1]], base=0,
                           channel_multiplier=V)
            flat_all = sm.tile([128, NT], i32, tag="flat")
            nc.vector.tensor_tensor(out=flat_all[:], in0=ids_all[:],
                                    in1=iota_t[:], op=Alu.add)
            nc.vector.tensor_tensor(out=flat_all[:], in0=flat_all[:],
                                    in1=iota_p[:].to_broadcast([128, NT]),
                                    op=Alu.add)
            for t in range(NT):
                # HW DGE honors only one index per partition per transfer
                nc.gpsimd.indirect_dma_start(
                    out=g_all[:, t:t + 1], out_offset=None, in_=logits_flat,
                    in_offset=bass.IndirectOffsetOnAxis(
                        ap=flat_all[:, t:t + 1], axis=0))
            # halo gather
            hrb = sm.tile([128, 1], i32, tag="hrb")
            nc.gpsimd.iota(hrb[:NHALO, :], pattern=[[1, 1]], base=512 * V,
                           channel_multiplier=V)
            hids = sm.tile([128, 1], i32, tag="hids")
            nc.gpsimd.dma_start(out=hids[:NHALO, :], in_=ids[512:EXT, :])
            hfl = sm.tile([128, 1], i32, tag="hfl")
            nc.vector.tensor_tensor(out=hfl[:NHALO, :], in0=hids[:NHALO, :],
                                    in1=hrb[:NHALO, :], op=Alu.add)
            nc.gpsimd.indirect_dma_start(
                out=g_all[:NHALO, NT:], out_offset=None, in_=logits_flat,
                in_offset=bass.IndirectOffsetOnAxis(ap=hfl[:NHALO, :1], axis=0))

            # ---- halo rows (2): vocab packed across partitions ----
            # layout [128, HF]: partition p = (row a=p//HQ, slice q=p%HQ)
            hx = lp.tile([128, HF], f32, tag="x")
            halo_src = bass.AP(logits.tensor, 512 * V,
                               [[V, NHALO], [HF, HQ], [1, HF]])
            nc.sync.dma_start(out=hx[:], in_=halo_src)
            hscr = scr.tile([128, HF], f32, tag="e")
            hsums = sm.tile([128, 1], f32, tag="hsums")
            nc.scalar.activation(out=hscr[:], in_=hx[:], func=Act.Exp,
                                 accum_out=hsums[:])
            hsel = big.tile([128, NHALO], f32, tag="hsel")
            nc.vector.memset(hsel[:], 0.0)
            for a in range(NHALO):
                nc.vector.memset(hsel[a * HQ:(a + 1) * HQ, a:a + 1], 1.0)
            psum_h = ps_o.tile([NHALO, 1], f32, tag="ph")
            nc.tensor.matmul(out=psum_h[:], lhsT=hsel[:], rhs=hsums[:],
                             start=True, stop=True)
            nc.vector.tensor_copy(out=se_all[:NHALO, NT:], in_=psum_h[:])

            # ---- pass 1: 8 exp chunks per main row tile, nothing else ----
            for t, (r0, pn) in enumerate(ROW_TILES):
                sums = sm.tile([128, NCH], f32, tag="sums")
                for ci in range(NCH):
                    x_sb = lp.tile([128, CF], f32, tag="x")
                    nc.sync.dma_start(
                        out=x_sb[:pn, :],
                        in_=logits[r0:r0 + pn, ci * CF:(ci + 1) * CF],
                    )
                    e_sb = scr.tile([128, CF], f32, tag="e")
                    nc.scalar.activation(
                        out=e_sb[:pn, :], in_=x_sb[:pn, :], func=Act.Exp,
                        accum_out=sums[:pn, ci:ci + 1],
                    )
                nc.vector.tensor_reduce(
                    out=se_all[:, t:t + 1], in_=sums[:, :],
                    axis=mybir.AxisListType.X, op=Alu.add,
                )

            # ---- batched LSE -> surp -> srow ----
            lse_all = sm.tile([128, NT + 1], f32, tag="lse")
            nc.scalar.activation(out=lse_all[:], in_=se_all[:], func=Act.Ln)
            surp_all = sm.tile([128, NT + 1], f32, tag="surp")
            nc.vector.tensor_tensor(out=surp_all[:], in0=lse_all[:],
                                    in1=g_all[:], op=Alu.subtract)
            nc.vector.tensor_tensor(out=surp_all[:], in0=surp_all[:],
                                    in1=m_all[:], op=Alu.mult)
            nc.vector.tensor_scalar(out=surp_all[:], in0=surp_all[:],
                                    scalar1=LOG2E, scalar2=None, op0=Alu.mult)
            srow = big.tile([1, EXT], f32, tag="srow")
            for t in range(NT):
                nc.gpsimd.dma_start(out=srow[0:1, 128 * t:128 * (t + 1)],
                                    in_=surp_all[:, t:t + 1])
            nc.gpsimd.dma_start(out=srow[0:1, 512:EXT],
                                in_=surp_all[:NHALO, NT:])

            # ---- surp channel: one contract-5 matmul closes the accumulation ----
            s5 = big.tile([K, Y_LOC], f32, tag="s5")
            for k in range(K):
                nc.gpsimd.dma_start(out=s5[k:k + 1, :],
                                    in_=srow[0:1, k:k + Y_LOC])
            nc.tensor.matmul(
                out=psum_y[:],
                lhsT=wsurp_sb[:],
                rhs=s5[:],
                start=False,
                stop=True,
            )

            # ---- maxpool(5) + bias + relu ----
            pooled = big.tile([OC, PO_LOC], f32, tag="pooled")
            stop_off = K * (PO_LOC - 1) + 1
            nc.vector.tensor_copy(out=pooled[:], in_=psum_y[:, 0:stop_off:K])
            for j in range(1, K):
                nc.vector.tensor_tensor(out=pooled[:], in0=pooled[:],
                                        in1=psum_y[:, j:j + stop_off:K], op=Alu.max)
            nc.vector.tensor_scalar(out=pooled[:], in0=pooled[:],
                                    scalar1=convb_sb[:, 0:1], scalar2=None,
                                    op0=Alu.add)
            nc.vector.tensor_scalar(out=pooled[:], in0=pooled[:],
                                    scalar1=0.0, scalar2=None, op0=Alu.max)

            # ---- FC partial: red[oc, l] = sum_p pooled*fcw ----
            red = big.tile([OC, 3], f32, tag="red")
            fc_scr = big.tile([OC, PO_LOC], f32, tag="fcscr")
            for l in range(3):
                nc.vector.tensor_tensor(
                    out=fc_scr[:],
                    in0=pooled[:],
                    in1=fcw_sb[:, l * PO_LOC:(l + 1) * PO_LOC],
                    op=Alu.mult,
                )
                nc.vector.tensor_reduce(
                    out=red[:, l:l + 1], in_=fc_scr[:],
                    axis=mybir.AxisListType.X, op=Alu.add,
                )
            # sentiment branch (zeroed on h==1 cores)
            rs = sm.tile([128, 1], f32, tag="rs")
            nc.vector.tensor_scalar(out=rs[:], in0=sentv_sb[:], scalar1=0.0,
                                    scalar2=None, op0=Alu.max)
            tmp3 = sm.tile([128, 3], f32, tag="tmp3")
            nc.vector.tensor_scalar(out=tmp3[:], in0=sentw_sb[:],
                                    scalar1=rs[:, 0:1], scalar2=None, op0=Alu.mult)
            nc.vector.tensor_tensor(out=red[:], in0=red[:], in1=tmp3[:], op=Alu.add)

            psum_out = ps_o.tile([3, 1], f32, tag="po")
            nc.tensor.matmul(out=psum_out[:], lhsT=red[:], rhs=ones_sb[:],
                             start=True, stop=True)
            out_sb = sm.tile([3, 1], f32, tag="outsb")
            nc.vector.tensor_tensor(out=out_sb[:], in0=psum_out[:], in1=fcb_sb[:],
                                    op=Alu.add)
            nc.sync.dma_start(out=out, in_=out_sb[:])

    nc.compile()
    return nc


def _prep_core_inputs(core, input_ids, attention_mask, sentiment, logits,
                      hidden, conv_w, conv_b, fc_w, fc_b):
    b, h = core // 2, core % 2
    g0 = Y_LOC * h
    ext0 = g0 - 2

    lg = np.zeros((EXT, V), np.float32)
    idl = np.zeros((EXT, 1), np.int32)
    mk = np.zeros((EXT, 1), np.float32)
    hd = np.zeros((EXT, H), np.float32)
    lo = max(0, -ext0)            # local index where valid rows start
    s0, s1 = ext0 + lo, ext0 + EXT
    lg[lo:] = logits[b, s0:s1]
    idl[lo:, 0] = input_ids[b, s0:s1].astype(np.int32)
    mk[lo:, 0] = attention_mask[b, s0:s1]
    hd[lo:] = hidden[b, s0:s1]

    wt = np.ascontiguousarray(
        conv_w[:, :H, :].transpose(1, 2, 0).reshape(H, K * OC))
    ws = np.ascontiguousarray(conv_w[:, H, :].T)           # [K, OC]
    cb = np.ascontiguousarray(conv_b[:, None])             # [OC, 1]

    w3 = fc_w[:, :OC * 204].reshape(3, OC, 204)
    fcw = np.ascontiguousarray(
        w3[:, :, h * PO_LOC:(h + 1) * PO_LOC].transpose(1, 0, 2).reshape(OC, 3 * PO_LOC))

    sv = np.zeros((128, 1), np.float32)
    sw = np.zeros((128, 3), np.float32)
    fb = np.zeros((3, 1), np.float32)
    if h == 0:
        sv[:3, 0] = sentiment[b]
        sw[:3, :] = fc_w[:, OC * 204:].T                   # [3 j, 3 l]
        fb[:, 0] = fc_b

    return {
        "logits_loc": lg, "ids_loc": idl, "mask_loc": mk, "hidden_loc": hd,
        "wt": wt, "wsurp": ws, "convb": cb, "fcw": fcw,
        "sentv": sv, "sentw": sw, "fcb": fb,
    }


def _install_ntff_hook():
    import sys
    import types
    try:
        import antenv
        from trn_agent_boot.trn_boot import _ntff_profile_via_ctypes
    except ImportError:
        return
    if "antenv.axon_hooks" in sys.modules:
        return
    mod = types.ModuleType("antenv.axon_hooks")
    _h = [None]
    mod.set_axon_ntff_profile_hook = lambda hk: _h.__setitem__(0, hk)
    mod.get_axon_ntff_profile_hook = lambda: _h[0]
    sys.modules["antenv.axon_hooks"] = mod
    antenv.axon_hooks = mod
    try:
        mod.set_axon_ntff_profile_hook(
            _ntff_profile_via_ctypes('/opt/axon/libaxon_pjrt.so'))
    except Exception:
        pass


def kernel(input_ids, attention_mask, sentiment, logits, hidden,
           conv_w, conv_b, fc_w, fc_b, _trace=False):
    from concourse.bass_utils import run_bass_kernel_spmd

    input_ids = np.asarray(input_ids)
    attention_mask = np.asarray(attention_mask, np.float32)
    sentiment = np.asarray(sentiment, np.float32)
    logits = np.asarray(logits, np.float32)
    hidden = np.asarray(hidden, np.float32)
    conv_w = np.asarray(conv_w, np.float32)
    conv_b = np.asarray(conv_b, np.float32)
    fc_w = np.asarray(fc_w, np.float32)
    fc_b = np.asarray(fc_b, np.float32)

    if "nc" not in _CACHE:
        _CACHE["nc"] = _build_program()
    nc = _CACHE["nc"]

    in_maps = [
        _prep_core_inputs(c, input_ids, attention_mask, sentiment, logits,
                          hidden, conv_w, conv_b, fc_w, fc_b)
        for c in range(N_CORES)
    ]
    if _trace:
        _install_ntff_hook()
    res = run_bass_kernel_spmd(nc, in_maps, list(range(N_CORES)), trace=_trace)
    _CACHE["last_result"] = res

    out = np.zeros((B, 3), np.float32)
    for b in range(B):
        out[b] = (res.results[2 * b]["out_loc"][:, 0]
                  + res.results[2 * b + 1]["out_loc"][:, 0])
    return out



# revision 9
# speedup vs baseline: 1.8387x; 1.8387x over previous
"""Trainium2 Bass kernel for nn_CNN_80221399155117.

Pipeline: full-vocab softmax -> token-prob gather -> -log2 surprisal ->
concat(hidden, surp) -> Conv1d(k=5, pad=2) -> MaxPool1d(5) -> ReLU -> FC.

Sharding: 8 cores = (batch b, seq-half h). Each core owns the pool-aligned
conv-output range [510h, 510h+510) of its batch, needing feats rows
[510h-2, 510h+512) (EXT=514, zero-padded outside [0,1024)). The softmax
normalizer is computed locally per row (positions sharded, vocab local),
so no collectives are needed.

v2: logits are host-cast to fp8(e4m3) -- 16.5 MB/core HBM stream instead
of 65.8 MB -- and the exp+sum work is split across the Scalar engine
(native Exp, vocab [0,18048)) and the Vector engine (custom fused DVE op
computing (1 + x/32)^32 ~ exp(x) with accumulate, vocab [18048,32000)).
The gathered token logit also comes from the fp8 tensor (quantization
noise shown to keep end-to-end rel err ~1.5e-3). Hidden is host-transposed
to [H, EXT] f16 so no on-device transposes are needed; conv weights are
host-cast f16.
"""

import numpy as np
import ml_dtypes

B, S, V, H = 4, 1024, 32000, 2048
OC, K = 128, 5
N_CORES = 8
Y_LOC = 510            # conv output positions per core (102 pool windows)
PO_LOC = 102           # pooled cols per core
EXT = 514              # feats rows incl conv halo (510 + 2 + 2)
LOG2E = 1.4426950408889634

# vocab split between engines: scalar 18048 (rate .833ns/el) vs DVE 13952
# (rate 1.042ns/el) -> both ~60us, under the ~70us DMA stream
SC_CH = [(0, 9024), (9024, 18048)]
DC_CH = [(18048, 25024), (25024, 32000)]
C0V = 1.03125 / 32.0   # tuned base for (1+c0*x)^32: cancels bulk of -x^2/64
COMP = -0.000195       # additive lse bias compensation (tuned by sim)

NP8 = ml_dtypes.float8_e4m3

_CACHE = {}


def _register_exp_sq():
    """Register the fused exp-approx+reduce custom DVE op (idempotent)."""
    from operator import add as _add
    import concourse.dve_ops as dvo
    from concourse.dve_spec import Spec, Src0, C0, C1, Zero, sq

    for op in dvo.OPS:
        if op.name == "EXP_SQ_REDUCE":
            return op

    def _ref(in0, in1, c0, c1, c2):
        t = (in0.astype(np.float32) * c0 + c1).astype(np.float32)
        for _ in range(5):
            t = (t * t).astype(np.float32)
        return t, t.reshape(t.shape[0], -1).sum(axis=-1, keepdims=True)

    body = Src0 * C0 + C1
    for _ in range(5):
        body = sq(body)
    op = dvo.DveOp(
        "EXP_SQ_REDUCE",
        Spec(body=body, accum=_add, accum_init=Zero, reference=_ref),
        subdim=False,
        uops_sha={"v3": "ea86ec6fb1475bcb"},
    )
    dvo._SUB_OPCODE_FOR_NAME["EXP_SQ_REDUCE"] = (
        max(dvo._SUB_OPCODE_FOR_NAME.values()) + 1)
    dvo.OPS.append(op)
    return op


def _build_program():
    import concourse.tile as tile
    from concourse import bacc, bass, mybir

    EXP_SQ = _register_exp_sq()

    f32 = mybir.dt.float32
    f16 = mybir.dt.float16
    fp8 = mybir.dt.float8e4
    i32 = mybir.dt.int32
    Alu = mybir.AluOpType
    Act = mybir.ActivationFunctionType

    nc = bacc.Bacc("TRN2", target_bir_lowering=False, debug=False,
                   num_devices=N_CORES)

    l8 = nc.dram_tensor("l8", [EXT, V], fp8, kind="ExternalInput").ap()
    ids = nc.dram_tensor("ids_loc", [EXT, 1], i32, kind="ExternalInput").ap()
    maskd = nc.dram_tensor("mask_loc", [EXT, 1], f32, kind="ExternalInput").ap()
    hidt = nc.dram_tensor("hidden_t", [H, EXT], f16, kind="ExternalInput").ap()
    wt = nc.dram_tensor("wt", [H, K * OC], f16, kind="ExternalInput").ap()
    wsurp = nc.dram_tensor("wsurp", [K, OC], f32, kind="ExternalInput").ap()
    convb = nc.dram_tensor("convb", [OC, 1], f32, kind="ExternalInput").ap()
    fcw = nc.dram_tensor("fcw", [OC, 3 * PO_LOC], f32, kind="ExternalInput").ap()
    sentv = nc.dram_tensor("sentv", [128, 1], f32, kind="ExternalInput").ap()
    sentw = nc.dram_tensor("sentw", [128, 3], f32, kind="ExternalInput").ap()
    fcb = nc.dram_tensor("fcb", [3, 1], f32, kind="ExternalInput").ap()
    out = nc.dram_tensor("out_loc", [3, 1], f32, kind="ExternalOutput").ap()

    l8_flat = bass.AP(l8.tensor, 0, [[1, EXT * V], [1, 1]])

    NT = 4                             # main row tiles of 128
    NHALO = EXT - 512                  # 2 halo rows, packed [128, HF]
    HQ = 128 // NHALO                  # partitions per halo row
    HF = V // HQ                       # free elems per partition

    with tile.TileContext(nc) as tc:
        with (
            tc.tile_pool(name="lps", bufs=3) as lps,        # scalar fp8 chunks
            tc.tile_pool(name="lpd", bufs=3) as lpd,        # dve fp8 chunks
            tc.tile_pool(name="scs", bufs=2) as scs,        # scalar exp scratch
            tc.tile_pool(name="scd", bufs=2) as scd,        # dve exp scratch
            tc.tile_pool(name="big", bufs=1) as big,        # resident
            tc.tile_pool(name="sm", bufs=12) as sm,         # small stats
            tc.tile_pool(name="ps_y", bufs=1, space="PSUM") as ps_y,
            tc.tile_pool(name="ps_h", bufs=1, space="PSUM") as ps_h,
        ):
            # ---- gather/stats setup (gpsimd smalls; runs while stream fills)
            ids_all = sm.tile([128, NT], i32, tag="ids")
            nc.gpsimd.dma_start(out=ids_all[:],
                                in_=bass.AP(ids.tensor, 0, [[1, 128], [128, NT]]))
            m_all = big.tile([128, NT + 1], f32, tag="m")
            nc.gpsimd.dma_start(out=m_all[:, :NT],
                                in_=bass.AP(maskd.tensor, 0, [[1, 128], [128, NT]]))
            nc.vector.memset(m_all[:, NT:], 0.0)
            nc.gpsimd.dma_start(out=m_all[:NHALO, NT:], in_=maskd[512:EXT, :])

            iota_t = sm.tile([128, NT], i32, tag="iota")
            nc.gpsimd.iota(iota_t[:], pattern=[[1, NT]], base=0,
                           channel_multiplier=0)
            nc.gpsimd.tensor_scalar(out=iota_t[:], in0=iota_t[:],
                                    scalar1=128 * V, scalar2=None, op0=Alu.mult)
            iota_p = sm.tile([128, 1], i32, tag="iotap")
            nc.gpsimd.iota(iota_p[:], pattern=[[1, 1]], base=0,
                           channel_multiplier=V)
            flat_all = sm.tile([128, NT], i32, tag="flat")
            nc.gpsimd.tensor_tensor(out=flat_all[:], in0=ids_all[:],
                                    in1=iota_t[:], op=Alu.add)
            nc.gpsimd.tensor_tensor(out=flat_all[:], in0=flat_all[:],
                                    in1=iota_p[:].to_broadcast([128, NT]),
                                    op=Alu.add)
            g8 = sm.tile([128, NT + 1], fp8, tag="g8")
            nc.vector.memset(g8[:], 0.0)
            for t in range(NT):
                # HW DGE honors only one index per partition per transfer
                nc.gpsimd.indirect_dma_start(
                    out=g8[:, t:t + 1], out_offset=None, in_=l8_flat,
                    in_offset=bass.IndirectOffsetOnAxis(
                        ap=flat_all[:, t:t + 1], axis=0))
            # halo gather
            hrb = sm.tile([128, 1], i32, tag="hrb")
            nc.gpsimd.iota(hrb[:NHALO, :], pattern=[[1, 1]], base=512 * V,
                           channel_multiplier=V)
            hids = sm.tile([128, 1], i32, tag="hids")
            nc.gpsimd.dma_start(out=hids[:NHALO, :], in_=ids[512:EXT, :])
            hfl = sm.tile([128, 1], i32, tag="hfl")
            nc.gpsimd.tensor_tensor(out=hfl[:NHALO, :], in0=hids[:NHALO, :],
                                    in1=hrb[:NHALO, :], op=Alu.add)
            nc.gpsimd.indirect_dma_start(
                out=g8[:NHALO, NT:], out_offset=None, in_=l8_flat,
                in_offset=bass.IndirectOffsetOnAxis(ap=hfl[:NHALO, :1], axis=0))
            g_all = big.tile([128, NT + 1], f32, tag="g")
            nc.vector.tensor_copy(out=g_all[:], in_=g8[:])

            # ---- halo rows (2): vocab packed across partitions ----
            se_all = big.tile([128, NT + 1], f32, tag="se")
            nc.vector.memset(se_all[:, NT:], 1.0)   # ln(1)=0 on unused lanes
            hx = sm.tile([128, HF], fp8, tag="hx")
            halo_src = bass.AP(l8.tensor, 512 * V,
                               [[V, NHALO], [HF, HQ], [1, HF]])
            nc.sync.dma_start(out=hx[:], in_=halo_src)
            hscr = sm.tile([128, HF], f16, tag="hscr")
            hsums = sm.tile([128, 1], f32, tag="hsums")
            nc.scalar.activation(out=hscr[:], in_=hx[:], func=Act.Exp,
                                 accum_out=hsums[:])
            hsel = big.tile([128, NHALO], f32, tag="hsel")
            nc.vector.memset(hsel[:], 0.0)
            for a in range(NHALO):
                nc.vector.memset(hsel[a * HQ:(a + 1) * HQ, a:a + 1], 1.0)
            psum_h = ps_h.tile([NHALO, 1], f32, tag="ph")
            nc.tensor.matmul(out=psum_h[:], lhsT=hsel[:], rhs=hsums[:],
                             start=True, stop=True)
            nc.vector.tensor_copy(out=se_all[:NHALO, NT:], in_=psum_h[:])

            # ---- main fp8 stream: scalar exp + DVE exp-approx, per row tile
            sums_s = big.tile([128, NT * len(SC_CH)], f32, tag="sums_s")
            sums_d = big.tile([128, NT * len(DC_CH)], f32, tag="sums_d")
            for t in range(NT):
                r0 = 128 * t
                for j, (c0, c1) in enumerate(SC_CH):
                    w = c1 - c0
                    x_sb = lps.tile([128, SC_CH[0][1]], fp8, tag="xs")
                    nc.sync.dma_start(out=x_sb[:, :w], in_=l8[r0:r0 + 128, c0:c1])
                    e_sb = scs.tile([128, SC_CH[0][1]], f16, tag="es")
                    nc.scalar.activation(
                        out=e_sb[:, :w], in_=x_sb[:, :w], func=Act.Exp,
                        accum_out=sums_s[:, t * len(SC_CH) + j: t * len(SC_CH) + j + 1])
                for j, (c0, c1) in enumerate(DC_CH):
                    w = c1 - c0
                    x_sb = lpd.tile([128, DC_CH[0][1] - DC_CH[0][0]], fp8, tag="xd")
                    nc.sync.dma_start(out=x_sb[:, :w], in_=l8[r0:r0 + 128, c0:c1])
                    e_sb = scd.tile([128, DC_CH[0][1] - DC_CH[0][0]], f16, tag="ed")
                    nc.vector._custom_dve(
                        EXP_SQ, out=e_sb[:, :w], in0=x_sb[:, :w],
                        s0=C0V, s1=1.0,
                        accum_out=sums_d[:, t * len(DC_CH) + j: t * len(DC_CH) + j + 1])

            # ---- resident weights / hidden_t (gpsimd DGE; needed mid-kernel)
            wtile = big.tile([128, 16 * K * OC], f16, tag="wtile")
            nc.gpsimd.dma_start(
                out=wtile[:].rearrange("p (c v) -> p c v", c=16),
                in_=wt.rearrange("(c p) v -> p c v", p=128))
            xt = big.tile([128, 16 * EXT], f16, tag="xt")
            nc.gpsimd.dma_start(
                out=xt[:].rearrange("p (c v) -> p c v", c=16),
                in_=hidt.rearrange("(c p) v -> p c v", p=128))
            wsurp_sb = big.tile([K, OC], f32, tag="wsurp")
            nc.gpsimd.dma_start(out=wsurp_sb[:], in_=wsurp)
            convb_sb = big.tile([OC, 1], f32, tag="convb")
            nc.gpsimd.dma_start(out=convb_sb[:], in_=convb)
            fcw_sb = big.tile([OC, 3 * PO_LOC], f32, tag="fcw")
            nc.gpsimd.dma_start(out=fcw_sb[:], in_=fcw)
            sentv_sb = big.tile([128, 1], f32, tag="sentv")
            nc.gpsimd.dma_start(out=sentv_sb[:], in_=sentv)
            sentw_sb = big.tile([128, 3], f32, tag="sentw")
            nc.gpsimd.dma_start(out=sentw_sb[:], in_=sentw)
            fcb_sb = big.tile([3, 1], f32, tag="fcb")
            nc.gpsimd.dma_start(out=fcb_sb[:], in_=fcb)

            # ---- conv: 80 hidden matmuls accumulate into one PSUM bank ----
            psum_y = ps_y.tile([OC, Y_LOC], f32, tag="y")
            first = True
            for cc in range(16):
                for k in range(K):
                    nc.tensor.matmul(
                        out=psum_y[:],
                        lhsT=wtile[:, cc * 640 + k * 128: cc * 640 + (k + 1) * 128],
                        rhs=xt[:, cc * EXT + k: cc * EXT + k + Y_LOC],
                        start=first,
                        stop=False,
                    )
                    first = False

            # ---- batched LSE -> surp -> srow ----
            nc.vector.tensor_reduce(
                out=se_all[:, 0:NT],
                in_=sums_s[:].rearrange("p (t j) -> p t j", t=NT),
                axis=mybir.AxisListType.X, op=Alu.add)
            sed = sm.tile([128, NT], f32, tag="sed")
            nc.vector.tensor_reduce(
                out=sed[:],
                in_=sums_d[:].rearrange("p (t j) -> p t j", t=NT),
                axis=mybir.AxisListType.X, op=Alu.add)
            nc.vector.tensor_tensor(out=se_all[:, 0:NT], in0=se_all[:, 0:NT],
                                    in1=sed[:], op=Alu.add)

            lse_all = sm.tile([128, NT + 1], f32, tag="lse")
            nc.scalar.activation(out=lse_all[:], in_=se_all[:], func=Act.Ln)
            surp_all = sm.tile([128, NT + 1], f32, tag="surp")
            nc.vector.tensor_tensor(out=surp_all[:], in0=lse_all[:],
                                    in1=g_all[:], op=Alu.subtract)
            nc.vector.tensor_scalar(out=surp_all[:], in0=surp_all[:],
                                    scalar1=COMP, op0=Alu.subtract,
                                    scalar2=LOG2E, op1=Alu.mult)
            nc.vector.tensor_tensor(out=surp_all[:], in0=surp_all[:],
                                    in1=m_all[:], op=Alu.mult)
            srow = big.tile([1, EXT], f32, tag="srow")
            for t in range(NT):
                nc.gpsimd.dma_start(out=srow[0:1, 128 * t:128 * (t + 1)],
                                    in_=surp_all[:, t:t + 1])
            nc.gpsimd.dma_start(out=srow[0:1, 512:EXT],
                                in_=surp_all[:NHALO, NT:])

            # ---- surp channel: one contract-5 matmul closes the accumulation
            s5 = big.tile([K, Y_LOC], f32, tag="s5")
            for k in range(K):
                nc.sync.dma_start(out=s5[k:k + 1, :],
                                  in_=srow[0:1, k:k + Y_LOC])
            nc.tensor.matmul(
                out=psum_y[:],
                lhsT=wsurp_sb[:],
                rhs=s5[:],
                start=False,
                stop=True,
            )

            # ---- maxpool(5) + bias + relu: two fused ops ----
            pooled = big.tile([OC, PO_LOC], f32, tag="pooled")
            nc.vector.tensor_reduce(
                out=pooled[:],
                in_=psum_y[:].rearrange("p (a b) -> p a b", b=K),
                axis=mybir.AxisListType.X, op=Alu.max)
            nc.vector.tensor_scalar(out=pooled[:], in0=pooled[:],
                                    scalar1=convb_sb[:, 0:1], op0=Alu.add,
                                    scalar2=0.0, op1=Alu.max)

            # ---- FC partial: red[oc, l] = sum_p pooled*fcw ----
            red = big.tile([OC, 3], f32, tag="red")
            fc_scr = big.tile([OC, PO_LOC], f32, tag="fcscr")
            for l in range(3):
                nc.vector.tensor_tensor(
                    out=fc_scr[:],
                    in0=pooled[:],
                    in1=fcw_sb[:, l * PO_LOC:(l + 1) * PO_LOC],
                    op=Alu.mult)
                nc.vector.tensor_reduce(
                    out=red[:, l:l + 1], in_=fc_scr[:],
                    axis=mybir.AxisListType.X, op=Alu.add)
            # sentiment branch (zeroed on h==1 cores)
            rs = sm.tile([128, 1], f32, tag="rs")
            nc.vector.tensor_scalar(out=rs[:], in0=sentv_sb[:], scalar1=0.0,
                                    scalar2=None, op0=Alu.max)
            tmp3 = sm.tile([128, 3], f32, tag="tmp3")
            nc.vector.tensor_scalar(out=tmp3[:], in0=sentw_sb[:],
                                    scalar1=rs[:, 0:1], scalar2=None,
                                    op0=Alu.mult)
            nc.vector.tensor_tensor(out=red[:], in0=red[:], in1=tmp3[:],
                                    op=Alu.add)

            ones_sb = big.tile([128, 1], f32, tag="ones")
            nc.vector.memset(ones_sb[:], 1.0)
            psum_out = ps_h.tile([3, 1], f32, tag="po")
            nc.tensor.matmul(out=psum_out[:], lhsT=red[:], rhs=ones_sb[:],
                             start=True, stop=True)
            out_sb = sm.tile([3, 1], f32, tag="outsb")
            nc.vector.tensor_tensor(out=out_sb[:], in0=psum_out[:],
                                    in1=fcb_sb[:], op=Alu.add)
            nc.sync.dma_start(out=out, in_=out_sb[:])

    nc.compile()
    return nc


def _prep_core_inputs(core, l8_full, input_ids, attention_mask, sentiment,
                      hidden, conv_w, conv_b, fc_w, fc_b):
    b, h = core // 2, core % 2
    g0 = Y_LOC * h
    ext0 = g0 - 2

    lg = np.zeros((EXT, V), NP8)
    idl = np.zeros((EXT, 1), np.int32)
    mk = np.zeros((EXT, 1), np.float32)
    hdt = np.zeros((H, EXT), np.float16)
    lo = max(0, -ext0)            # local index where valid rows start
    s0, s1 = ext0 + lo, ext0 + EXT
    lg[lo:] = l8_full[b, s0:s1]
    idl[lo:, 0] = input_ids[b, s0:s1].astype(np.int32)
    mk[lo:, 0] = attention_mask[b, s0:s1]
    hdt[:, lo:] = hidden[b, s0:s1].T.astype(np.float16)

    wt = np.ascontiguousarray(
        conv_w[:, :H, :].transpose(1, 2, 0).reshape(H, K * OC)).astype(np.float16)
    ws = np.ascontiguousarray(conv_w[:, H, :].T)           # [K, OC]
    cb = np.ascontiguousarray(conv_b[:, None])             # [OC, 1]

    w3 = fc_w[:, :OC * 204].reshape(3, OC, 204)
    fcw = np.ascontiguousarray(
        w3[:, :, h * PO_LOC:(h + 1) * PO_LOC].transpose(1, 0, 2).reshape(OC, 3 * PO_LOC))

    sv = np.zeros((128, 1), np.float32)
    sw = np.zeros((128, 3), np.float32)
    fb = np.zeros((3, 1), np.float32)
    if h == 0:
        sv[:3, 0] = sentiment[b]
        sw[:3, :] = fc_w[:, OC * 204:].T                   # [3 j, 3 l]
        fb[:, 0] = fc_b

    return {
        "l8": lg, "ids_loc": idl, "mask_loc": mk, "hidden_t": hdt,
        "wt": wt, "wsurp": ws, "convb": cb, "fcw": fcw,
        "sentv": sv, "sentw": sw, "fcb": fb,
    }


def _install_ntff_hook():
    import sys
    import types
    try:
        import antenv
        from trn_agent_boot.trn_boot import _ntff_profile_via_ctypes
    except ImportError:
        return
    if "antenv.axon_hooks" in sys.modules:
        return
    mod = types.ModuleType("antenv.axon_hooks")
    _h = [None]
    mod.set_axon_ntff_profile_hook = lambda hk: _h.__setitem__(0, hk)
    mod.get_axon_ntff_profile_hook = lambda: _h[0]
    sys.modules["antenv.axon_hooks"] = mod
    antenv.axon_hooks = mod
    try:
        mod.set_axon_ntff_profile_hook(
            _ntff_profile_via_ctypes('/opt/axon/libaxon_pjrt.so'))
    except Exception:
        pass


def kernel(input_ids, attention_mask, sentiment, logits, hidden,
           conv_w, conv_b, fc_w, fc_b, _trace=False):
    from concourse.bass_utils import run_bass_kernel_spmd

    input_ids = np.asarray(input_ids)
    attention_mask = np.asarray(attention_mask, np.float32)
    sentiment = np.asarray(sentiment, np.float32)
    logits = np.asarray(logits, np.float32)
    hidden = np.asarray(hidden, np.float32)
    conv_w = np.asarray(conv_w, np.float32)
    conv_b = np.asarray(conv_b, np.float32)
    fc_w = np.asarray(fc_w, np.float32)
    fc_b = np.asarray(fc_b, np.float32)

    if "nc" not in _CACHE:
        _CACHE["nc"] = _build_program()
    nc = _CACHE["nc"]

    l8_full = logits.astype(NP8)       # one cast, sliced per core below
    in_maps = [
        _prep_core_inputs(c, l8_full, input_ids, attention_mask, sentiment,
                          hidden, conv_w, conv_b, fc_w, fc_b)
        for c in range(N_CORES)
    ]
    if _trace:
        _install_ntff_hook()
    res = run_bass_kernel_spmd(nc, in_maps, list(range(N_CORES)), trace=_trace)
    _CACHE["last_result"] = res

    out = np.zeros((B, 3), np.float32)
    for b in range(B):
        out[b] = (res.results[2 * b]["out_loc"][:, 0]
                  + res.results[2 * b + 1]["out_loc"][:, 0])
    return out


# revision 12
# speedup vs baseline: 2.2042x; 1.1988x over previous
"""Trainium2 Bass kernel for nn_CNN_80221399155117.

Pipeline: full-vocab softmax -> token-prob gather -> -log2 surprisal ->
concat(hidden, surp) -> Conv1d(k=5, pad=2) -> MaxPool1d(5) -> ReLU -> FC.

Sharding: 8 cores = (batch b, seq-half h). Each core owns the pool-aligned
conv-output range [510h, 510h+510) of its batch, needing feats rows
[510h-2, 510h+512) (EXT=514, zero-padded outside [0,1024)). The softmax
normalizer is computed locally per row (positions sharded, vocab local),
so no collectives are needed.

v2: logits are host-cast to fp8(e4m3) -- 16.5 MB/core HBM stream instead
of 65.8 MB -- and the exp+sum work is split across the Scalar engine
(native Exp, vocab [0,18048)) and the Vector engine (custom fused DVE op
computing (1 + x/32)^32 ~ exp(x) with accumulate, vocab [18048,32000)).
The gathered token logit also comes from the fp8 tensor (quantization
noise shown to keep end-to-end rel err ~1.5e-3). Hidden is host-transposed
to [H, EXT] f16 so no on-device transposes are needed; conv weights are
host-cast f16.
"""

import numpy as np
import ml_dtypes

B, S, V, H = 4, 1024, 32000, 2048
OC, K = 128, 5
N_CORES = 8
Y_LOC = 510            # conv output positions per core (102 pool windows)
PO_LOC = 102           # pooled cols per core
EXT = 514              # feats rows incl conv halo (510 + 2 + 2)
LOG2E = 1.4426950408889634

# vocab split between engines: scalar 18048 (measured 1.03ns/el) vs DVE
# 13952 (measured 1.27ns/el) -> both ~62us alongside the ~70us DMA stream.
# One chunk per row tile; last tile split in two to shorten the drain.
VS = 18048
SCH = [(0, 0, VS), (1, 0, VS), (2, 0, VS), (3, 0, VS // 2), (3, VS // 2, VS)]
DCH = [(0, VS, V), (1, VS, V), (2, VS, V),
       (3, VS, (VS + V) // 2), (3, (VS + V) // 2, V)]
C0V = 1.03125 / 32.0   # tuned base for (1+c0*x)^32: cancels bulk of -x^2/64
COMP = -0.000195       # additive lse bias compensation (tuned by sim)

NP8 = ml_dtypes.float8_e4m3

_CACHE = {}


def _register_exp_sq():
    """Register the fused exp-approx+reduce custom DVE op (idempotent)."""
    from operator import add as _add
    import concourse.dve_ops as dvo
    from concourse.dve_spec import Spec, Src0, C0, C1, Zero, sq

    for op in dvo.OPS:
        if op.name == "EXP_SQ_REDUCE":
            return op

    def _ref(in0, in1, c0, c1, c2):
        t = (in0.astype(np.float32) * c0 + c1).astype(np.float32)
        for _ in range(5):
            t = (t * t).astype(np.float32)
        return t, t.reshape(t.shape[0], -1).sum(axis=-1, keepdims=True)

    body = Src0 * C0 + C1
    for _ in range(5):
        body = sq(body)
    op = dvo.DveOp(
        "EXP_SQ_REDUCE",
        Spec(body=body, accum=_add, accum_init=Zero, reference=_ref),
        subdim=False,
        uops_sha={"v3": "ea86ec6fb1475bcb"},
    )
    dvo._SUB_OPCODE_FOR_NAME["EXP_SQ_REDUCE"] = (
        max(dvo._SUB_OPCODE_FOR_NAME.values()) + 1)
    dvo.OPS.append(op)
    return op


def _build_program():
    import concourse.tile as tile
    from concourse import bacc, bass, mybir

    EXP_SQ = _register_exp_sq()

    f32 = mybir.dt.float32
    f16 = mybir.dt.float16
    fp8 = mybir.dt.float8e4
    i32 = mybir.dt.int32
    Alu = mybir.AluOpType
    Act = mybir.ActivationFunctionType

    nc = bacc.Bacc("TRN2", target_bir_lowering=False, debug=False,
                   num_devices=N_CORES)

    l8 = nc.dram_tensor("l8", [EXT, V], fp8, kind="ExternalInput").ap()
    ids = nc.dram_tensor("ids_loc", [EXT, 1], i32, kind="ExternalInput").ap()
    maskd = nc.dram_tensor("mask_loc", [EXT, 1], f32, kind="ExternalInput").ap()
    hidt = nc.dram_tensor("hidden_t", [H, EXT], f16, kind="ExternalInput").ap()
    wt = nc.dram_tensor("wt", [H, K * OC], f16, kind="ExternalInput").ap()
    wsurp = nc.dram_tensor("wsurp", [K, OC], f16, kind="ExternalInput").ap()
    convb = nc.dram_tensor("convb", [OC, 1], f32, kind="ExternalInput").ap()
    fcw = nc.dram_tensor("fcw", [OC, 3 * PO_LOC], f32, kind="ExternalInput").ap()
    sentv = nc.dram_tensor("sentv", [128, 1], f32, kind="ExternalInput").ap()
    sentw = nc.dram_tensor("sentw", [128, 3], f32, kind="ExternalInput").ap()
    fcb = nc.dram_tensor("fcb", [3, 1], f32, kind="ExternalInput").ap()
    out = nc.dram_tensor("out_loc", [3, 1], f32, kind="ExternalOutput").ap()

    l8_flat = bass.AP(l8.tensor, 0, [[1, EXT * V], [1, 1]])

    NT = 4                             # main row tiles of 128
    NHALO = EXT - 512                  # 2 halo rows, packed [128, HF]
    HQ = 128 // NHALO                  # partitions per halo row
    HF = V // HQ                       # free elems per partition

    with tile.TileContext(nc) as tc:
        with (
            tc.tile_pool(name="lps", bufs=2) as lps,        # scalar fp8 chunks
            tc.tile_pool(name="lpd", bufs=2) as lpd,        # dve fp8 chunks
            tc.tile_pool(name="scs", bufs=1) as scs,        # scalar exp scratch
            tc.tile_pool(name="scd", bufs=1) as scd,        # dve exp scratch
            tc.tile_pool(name="big", bufs=1) as big,        # resident
            tc.tile_pool(name="sm", bufs=12) as sm,         # small stats
            tc.tile_pool(name="ps_y", bufs=1, space="PSUM") as ps_y,
            tc.tile_pool(name="ps_h", bufs=1, space="PSUM") as ps_h,
        ):
            # ---- scalar ring: 2 chunk DMAs ahead, then exp pipeline ----
            sums_s = big.tile([128, len(SCH)], f32, tag="sums_s")
            sums_d = big.tile([128, len(DCH)], f32, tag="sums_d")
            xs_tiles = []
            for k in range(2):
                t, c0, c1 = SCH[k]
                x_sb = lps.tile([128, VS], fp8, tag="xs")
                nc.scalar.dma_start(out=x_sb[:, :c1 - c0],
                                    in_=l8[128 * t:128 * t + 128, c0:c1])
                xs_tiles.append(x_sb)
            xd_tiles = []
            for k in range(2):
                t, c0, c1 = DCH[k]
                x_sb = lpd.tile([128, V - VS], fp8, tag="xd")
                nc.sync.dma_start(out=x_sb[:, :c1 - c0],
                                  in_=l8[128 * t:128 * t + 128, c0:c1])
                xd_tiles.append(x_sb)

            # ---- halo rows (2): vocab packed across partitions ----
            hx = sm.tile([128, HF], fp8, tag="hx")
            halo_src = bass.AP(l8.tensor, 512 * V,
                               [[V, NHALO], [HF, HQ], [1, HF]])
            nc.sync.dma_start(out=hx[:], in_=halo_src)
            hscr = sm.tile([128, HF], f16, tag="hscr")
            hsums = sm.tile([128, 1], f32, tag="hsums")
            nc.scalar.activation(out=hscr[:], in_=hx[:], func=Act.Exp,
                                 accum_out=hsums[:])

            # ---- gather/stats setup (gpsimd smalls + indirect gathers) ----
            ids_all = sm.tile([128, NT], i32, tag="ids")
            nc.gpsimd.dma_start(out=ids_all[:],
                                in_=bass.AP(ids.tensor, 0, [[1, 128], [128, NT]]))
            m_all = big.tile([128, NT + 1], f32, tag="m")
            nc.gpsimd.dma_start(out=m_all[:, :NT],
                                in_=bass.AP(maskd.tensor, 0, [[1, 128], [128, NT]]))
            nc.gpsimd.memset(m_all[:, NT:], 0.0)
            nc.gpsimd.dma_start(out=m_all[:NHALO, NT:], in_=maskd[512:EXT, :])
            se_all = big.tile([128, NT + 1], f32, tag="se")
            nc.gpsimd.memset(se_all[:, NT:], 1.0)   # ln(1)=0 on unused lanes
            hsel = big.tile([128, NHALO], f32, tag="hsel")
            nc.gpsimd.memset(hsel[:], 0.0)
            for a in range(NHALO):
                nc.gpsimd.memset(hsel[a * HQ:(a + 1) * HQ, a:a + 1], 1.0)

            iota_t = sm.tile([128, NT], i32, tag="iota")
            nc.gpsimd.iota(iota_t[:], pattern=[[1, NT]], base=0,
                           channel_multiplier=0)
            nc.gpsimd.tensor_scalar(out=iota_t[:], in0=iota_t[:],
                                    scalar1=128 * V, scalar2=None, op0=Alu.mult)
            iota_p = sm.tile([128, 1], i32, tag="iotap")
            nc.gpsimd.iota(iota_p[:], pattern=[[1, 1]], base=0,
                           channel_multiplier=V)
            flat_all = sm.tile([128, NT], i32, tag="flat")
            nc.gpsimd.tensor_tensor(out=flat_all[:], in0=ids_all[:],
                                    in1=iota_t[:], op=Alu.add)
            nc.gpsimd.tensor_tensor(out=flat_all[:], in0=flat_all[:],
                                    in1=iota_p[:].to_broadcast([128, NT]),
                                    op=Alu.add)
            g8 = sm.tile([128, NT + 1], fp8, tag="g8")
            nc.gpsimd.memset(g8[:], 0.0)
            for t in range(NT):
                # HW DGE honors only one index per partition per transfer
                nc.gpsimd.indirect_dma_start(
                    out=g8[:, t:t + 1], out_offset=None, in_=l8_flat,
                    in_offset=bass.IndirectOffsetOnAxis(
                        ap=flat_all[:, t:t + 1], axis=0))
            # halo gather
            hrb = sm.tile([128, 1], i32, tag="hrb")
            nc.gpsimd.iota(hrb[:NHALO, :], pattern=[[1, 1]], base=512 * V,
                           channel_multiplier=V)
            hids = sm.tile([128, 1], i32, tag="hids")
            nc.gpsimd.dma_start(out=hids[:NHALO, :], in_=ids[512:EXT, :])
            hfl = sm.tile([128, 1], i32, tag="hfl")
            nc.gpsimd.tensor_tensor(out=hfl[:NHALO, :], in0=hids[:NHALO, :],
                                    in1=hrb[:NHALO, :], op=Alu.add)
            nc.gpsimd.indirect_dma_start(
                out=g8[:NHALO, NT:], out_offset=None, in_=l8_flat,
                in_offset=bass.IndirectOffsetOnAxis(ap=hfl[:NHALO, :1], axis=0))

            # halo partition-reduce via select matmul (PE)
            psum_h = ps_h.tile([NHALO, 1], f32, tag="ph")
            nc.tensor.matmul(out=psum_h[:], lhsT=hsel[:], rhs=hsums[:],
                             start=True, stop=True)

            # small resident weights (sync ring; tiny)
            wsurp_sb = big.tile([K, OC], f16, tag="wsurp")
            nc.sync.dma_start(out=wsurp_sb[:], in_=wsurp)
            convb_sb = big.tile([OC, 1], f32, tag="convb")
            nc.sync.dma_start(out=convb_sb[:], in_=convb)
            fcw_sb = big.tile([OC, 3 * PO_LOC], f32, tag="fcw")
            nc.sync.dma_start(out=fcw_sb[:], in_=fcw)
            sentv_sb = big.tile([128, 1], f32, tag="sentv")
            nc.sync.dma_start(out=sentv_sb[:], in_=sentv)
            sentw_sb = big.tile([128, 3], f32, tag="sentw")
            nc.sync.dma_start(out=sentw_sb[:], in_=sentw)
            fcb_sb = big.tile([3, 1], f32, tag="fcb")
            nc.sync.dma_start(out=fcb_sb[:], in_=fcb)

            # ---- main stream: self-issued DMAs interleaved with exp ----
            wtile = big.tile([128, 16 * K * OC], f16, tag="wtile")
            xt = big.tile([128, 16 * EXT], f16, tag="xt")
            es = scs.tile([128, VS], f16, tag="es")
            ed = scd.tile([128, V - VS], f16, tag="ed")
            for k in range(len(SCH)):
                # scalar: exp chunk k; prefetch chunk k+2 afterwards
                t, c0, c1 = SCH[k]
                nc.scalar.activation(
                    out=es[:, :c1 - c0], in_=xs_tiles[k][:, :c1 - c0],
                    func=Act.Exp, accum_out=sums_s[:, k:k + 1])
                if k == 0:
                    # big resident loads issue here: queues land them by
                    # ~mid-stream, well before the conv matmuls need them
                    nc.scalar.dma_start(
                        out=wtile[:].rearrange("p (c v) -> p c v", c=16),
                        in_=wt.rearrange("(c p) v -> p c v", p=128))
                    nc.scalar.dma_start(
                        out=xt[:].rearrange("p (c v) -> p c v", c=16),
                        in_=hidt.rearrange("(c p) v -> p c v", p=128))
                if k + 2 < len(SCH):
                    t2, d0, d1 = SCH[k + 2]
                    x_sb = lps.tile([128, VS], fp8, tag="xs")
                    nc.scalar.dma_start(out=x_sb[:, :d1 - d0],
                                        in_=l8[128 * t2:128 * t2 + 128, d0:d1])
                    xs_tiles.append(x_sb)
                # vector: same pattern
                t, c0, c1 = DCH[k]
                nc.vector._custom_dve(
                    EXP_SQ, out=ed[:, :c1 - c0], in0=xd_tiles[k][:, :c1 - c0],
                    s0=C0V, s1=1.0, accum_out=sums_d[:, k:k + 1])
                if k + 2 < len(DCH):
                    t2, d0, d1 = DCH[k + 2]
                    x_sb = lpd.tile([128, V - VS], fp8, tag="xd")
                    nc.sync.dma_start(out=x_sb[:, :d1 - d0],
                                      in_=l8[128 * t2:128 * t2 + 128, d0:d1])
                    xd_tiles.append(x_sb)

            # ---- conv: 80 hidden matmuls accumulate into one PSUM bank ----
            psum_y = ps_y.tile([OC, Y_LOC], f32, tag="y")
            first = True
            for cc in range(16):
                for k in range(K):
                    nc.tensor.matmul(
                        out=psum_y[:],
                        lhsT=wtile[:, cc * 640 + k * 128: cc * 640 + (k + 1) * 128],
                        rhs=xt[:, cc * EXT + k: cc * EXT + k + Y_LOC],
                        start=first,
                        stop=False,
                    )
                    first = False

            # ---- batched LSE -> surp -> srow ----
            nc.vector.tensor_copy(out=se_all[:NHALO, NT:], in_=psum_h[:])
            g_all = big.tile([128, NT + 1], f32, tag="g")
            nc.vector.tensor_copy(out=g_all[:], in_=g8[:])
            # se per tile: cols [t0,t1,t2,t3a,t3b]
            nc.vector.tensor_tensor(out=se_all[:, 0:3], in0=sums_s[:, 0:3],
                                    in1=sums_d[:, 0:3], op=Alu.add)
            t3p = sm.tile([128, 2], f32, tag="t3p")
            nc.vector.tensor_tensor(out=t3p[:], in0=sums_s[:, 3:5],
                                    in1=sums_d[:, 3:5], op=Alu.add)
            nc.vector.tensor_reduce(out=se_all[:, 3:4], in_=t3p[:],
                                    axis=mybir.AxisListType.X, op=Alu.add)

            lse_all = sm.tile([128, NT + 1], f32, tag="lse")
            nc.scalar.activation(out=lse_all[:], in_=se_all[:], func=Act.Ln)
            surp_all = sm.tile([128, NT + 1], f32, tag="surp")
            nc.vector.tensor_tensor(out=surp_all[:], in0=lse_all[:],
                                    in1=g_all[:], op=Alu.subtract)
            nc.vector.tensor_scalar(out=surp_all[:], in0=surp_all[:],
                                    scalar1=COMP, op0=Alu.subtract,
                                    scalar2=LOG2E, op1=Alu.mult)
            nc.vector.tensor_tensor(out=surp_all[:], in0=surp_all[:],
                                    in1=m_all[:], op=Alu.mult)
            # srow (f16): one strided DMA for the 512 main rows + halo pair
            srow = big.tile([1, EXT], f16, tag="srow")
            for t in range(NT):
                nc.gpsimd.dma_start(out=srow[0:1, 128 * t:128 * (t + 1)],
                                    in_=surp_all[:, t:t + 1])
            nc.gpsimd.dma_start(out=srow[0:1, 512:EXT],
                                in_=surp_all[:NHALO, NT:])

            # ---- surp channel: one contract-5 matmul closes the accumulation
            s5 = big.tile([K, Y_LOC], f16, tag="s5")
            s5_src = bass.AP(srow[:].tensor, srow[:].offset,
                             [[1, 1], [1, K], [1, Y_LOC]])
            nc.sync.dma_start(out=s5[:], in_=s5_src)
            nc.tensor.matmul(
                out=psum_y[:],
                lhsT=wsurp_sb[:],
                rhs=s5[:],
                start=False,
                stop=True,
            )

            # ---- maxpool(5) + bias + relu: two fused ops ----
            pooled = big.tile([OC, PO_LOC], f32, tag="pooled")
            nc.vector.tensor_reduce(
                out=pooled[:],
                in_=psum_y[:].rearrange("p (a b) -> p a b", b=K),
                axis=mybir.AxisListType.X, op=Alu.max)
            nc.vector.tensor_scalar(out=pooled[:], in0=pooled[:],
                                    scalar1=convb_sb[:, 0:1], op0=Alu.add,
                                    scalar2=0.0, op1=Alu.max)

            # ---- FC partial: red[oc, l] = sum_p pooled*fcw ----
            red = big.tile([OC, 3], f32, tag="red")
            fc_scr = big.tile([OC, PO_LOC], f32, tag="fcscr")
            for l in range(3):
                nc.vector.tensor_tensor(
                    out=fc_scr[:],
                    in0=pooled[:],
                    in1=fcw_sb[:, l * PO_LOC:(l + 1) * PO_LOC],
                    op=Alu.mult)
                nc.vector.tensor_reduce(
                    out=red[:, l:l + 1], in_=fc_scr[:],
                    axis=mybir.AxisListType.X, op=Alu.add)
            # sentiment branch (zeroed on h==1 cores)
            rs = sm.tile([128, 1], f32, tag="rs")
            nc.vector.tensor_scalar(out=rs[:], in0=sentv_sb[:], scalar1=0.0,
                                    scalar2=None, op0=Alu.max)
            tmp3 = sm.tile([128, 3], f32, tag="tmp3")
            nc.vector.tensor_scalar(out=tmp3[:], in0=sentw_sb[:],
                                    scalar1=rs[:, 0:1], scalar2=None,
                                    op0=Alu.mult)
            nc.vector.tensor_tensor(out=red[:], in0=red[:], in1=tmp3[:],
                                    op=Alu.add)

            ones_sb = big.tile([128, 1], f32, tag="ones")
            nc.vector.memset(ones_sb[:], 1.0)
            psum_out = ps_h.tile([3, 1], f32, tag="po")
            nc.tensor.matmul(out=psum_out[:], lhsT=red[:], rhs=ones_sb[:],
                             start=True, stop=True)
            out_sb = sm.tile([3, 1], f32, tag="outsb")
            nc.vector.tensor_tensor(out=out_sb[:], in0=psum_out[:],
                                    in1=fcb_sb[:], op=Alu.add)
            nc.sync.dma_start(out=out, in_=out_sb[:])

    nc.compile()
    return nc


def _prep_core_inputs(core, l8_full, input_ids, attention_mask, sentiment,
                      hidden, conv_w, conv_b, fc_w, fc_b):
    b, h = core // 2, core % 2
    g0 = Y_LOC * h
    ext0 = g0 - 2

    lg = np.zeros((EXT, V), NP8)
    idl = np.zeros((EXT, 1), np.int32)
    mk = np.zeros((EXT, 1), np.float32)
    hdt = np.zeros((H, EXT), np.float16)
    lo = max(0, -ext0)            # local index where valid rows start
    s0, s1 = ext0 + lo, ext0 + EXT
    lg[lo:] = l8_full[b, s0:s1]
    idl[lo:, 0] = input_ids[b, s0:s1].astype(np.int32)
    mk[lo:, 0] = attention_mask[b, s0:s1]
    hdt[:, lo:] = hidden[b, s0:s1].T.astype(np.float16)

    wt = np.ascontiguousarray(
        conv_w[:, :H, :].transpose(1, 2, 0).reshape(H, K * OC)).astype(np.float16)
    ws = np.ascontiguousarray(conv_w[:, H, :].T).astype(np.float16)  # [K, OC]
    cb = np.ascontiguousarray(conv_b[:, None])             # [OC, 1]

    w3 = fc_w[:, :OC * 204].reshape(3, OC, 204)
    fcw = np.ascontiguousarray(
        w3[:, :, h * PO_LOC:(h + 1) * PO_LOC].transpose(1, 0, 2).reshape(OC, 3 * PO_LOC))

    sv = np.zeros((128, 1), np.float32)
    sw = np.zeros((128, 3), np.float32)
    fb = np.zeros((3, 1), np.float32)
    if h == 0:
        sv[:3, 0] = sentiment[b]
        sw[:3, :] = fc_w[:, OC * 204:].T                   # [3 j, 3 l]
        fb[:, 0] = fc_b

    return {
        "l8": lg, "ids_loc": idl, "mask_loc": mk, "hidden_t": hdt,
        "wt": wt, "wsurp": ws, "convb": cb, "fcw": fcw,
        "sentv": sv, "sentw": sw, "fcb": fb,
    }


def _install_ntff_hook():
    import sys
    import types
    try:
        import antenv
        from trn_agent_boot.trn_boot import _ntff_profile_via_ctypes
    except ImportError:
        return
    if "antenv.axon_hooks" in sys.modules:
        return
    mod = types.ModuleType("antenv.axon_hooks")
    _h = [None]
    mod.set_axon_ntff_profile_hook = lambda hk: _h.__setitem__(0, hk)
    mod.get_axon_ntff_profile_hook = lambda: _h[0]
    sys.modules["antenv.axon_hooks"] = mod
    antenv.axon_hooks = mod
    try:
        mod.set_axon_ntff_profile_hook(
            _ntff_profile_via_ctypes('/opt/axon/libaxon_pjrt.so'))
    except Exception:
        pass


def kernel(input_ids, attention_mask, sentiment, logits, hidden,
           conv_w, conv_b, fc_w, fc_b, _trace=False):
    from concourse.bass_utils import run_bass_kernel_spmd

    input_ids = np.asarray(input_ids)
    attention_mask = np.asarray(attention_mask, np.float32)
    sentiment = np.asarray(sentiment, np.float32)
    logits = np.asarray(logits, np.float32)
    hidden = np.asarray(hidden, np.float32)
    conv_w = np.asarray(conv_w, np.float32)
    conv_b = np.asarray(conv_b, np.float32)
    fc_w = np.asarray(fc_w, np.float32)
    fc_b = np.asarray(fc_b, np.float32)

    if "nc" not in _CACHE:
        _CACHE["nc"] = _build_program()
    nc = _CACHE["nc"]

    l8_full = logits.astype(NP8)       # one cast, sliced per core below
    in_maps = [
        _prep_core_inputs(c, l8_full, input_ids, attention_mask, sentiment,
                          hidden, conv_w, conv_b, fc_w, fc_b)
        for c in range(N_CORES)
    ]
    if _trace:
        _install_ntff_hook()
    res = run_bass_kernel_spmd(nc, in_maps, list(range(N_CORES)), trace=_trace)
    _CACHE["last_result"] = res

    out = np.zeros((B, 3), np.float32)
    for b in range(B):
        out[b] = (res.results[2 * b]["out_loc"][:, 0]
                  + res.results[2 * b + 1]["out_loc"][:, 0])
    return out


# revision 14
# speedup vs baseline: 2.2787x; 1.0338x over previous
"""Trainium2 Bass kernel for nn_CNN_80221399155117.

Pipeline: full-vocab softmax -> token-prob gather -> -log2 surprisal ->
concat(hidden, surp) -> Conv1d(k=5, pad=2) -> MaxPool1d(5) -> ReLU -> FC.

Sharding: 8 cores = (batch b, seq-half h). Each core owns the pool-aligned
conv-output range [510h, 510h+510) of its batch, needing feats rows
[510h-2, 510h+512) (EXT=514, zero-padded outside [0,1024)). The softmax
normalizer is computed locally per row (positions sharded, vocab local),
so no collectives are needed.

v2: logits are host-cast to fp8(e4m3) -- 16.5 MB/core HBM stream instead
of 65.8 MB -- and the exp+sum work is split across the Scalar engine
(native Exp, vocab [0,18048)) and the Vector engine (custom fused DVE op
computing (1 + x/32)^32 ~ exp(x) with accumulate, vocab [18048,32000)).
The gathered token logit also comes from the fp8 tensor (quantization
noise shown to keep end-to-end rel err ~1.5e-3). Hidden is host-transposed
to [H, EXT] f16 so no on-device transposes are needed; conv weights are
host-cast f16.
"""

import numpy as np
import ml_dtypes

B, S, V, H = 4, 1024, 32000, 2048
OC, K = 128, 5
N_CORES = 8
Y_LOC = 510            # conv output positions per core (102 pool windows)
PO_LOC = 102           # pooled cols per core
EXT = 514              # feats rows incl conv halo (510 + 2 + 2)
LOG2E = 1.4426950408889634

# vocab split between engines: scalar [0,17664) (measured ~0.85ns/el) vs
# DVE [17664,32000) (measured ~1.05ns/el) -> both ~60us alongside the
# ~70us fp8 DMA stream. Tile 0 is tapered into small sub-chunks so the
# first exp starts ~4us in (equal-sized prefetches all round-robin
# through the queues and would otherwise finish together at ~27us);
# tile 3 is halved to shorten the drain. Each chunk owns an accumulator
# column in a [128, 16] sums tile (tile t -> cols 4t..), pre-zeroed, so
# per-tile sums are one 3D reduce.
VS = 17664
DW = V - VS
def _taper(base, width):
    q = width // 8
    return [(0, base, base + q), (0, base + q, base + 2 * q),
            (0, base + 2 * q, base + 4 * q), (0, base + 4 * q, base + width)]
SCH = (_taper(0, VS) + [(1, 0, VS), (2, 0, VS),
                        (3, 0, VS // 2), (3, VS // 2, VS)])
DCH = (_taper(VS, DW) + [(1, VS, V), (2, VS, V),
                         (3, VS, VS + DW // 2), (3, VS + DW // 2, V)])
SCOL = [0, 1, 2, 3, 4, 8, 12, 13]
C0V = 1.03125 / 32.0   # tuned base for (1+c0*x)^32: cancels bulk of -x^2/64
COMP = -0.000195       # additive lse bias compensation (tuned by sim)

NP8 = ml_dtypes.float8_e4m3

_CACHE = {}


def _register_exp_sq():
    """Register the fused exp-approx+reduce custom DVE op (idempotent)."""
    from operator import add as _add
    import concourse.dve_ops as dvo
    from concourse.dve_spec import Spec, Src0, C0, C1, Zero, sq

    for op in dvo.OPS:
        if op.name == "EXP_SQ_REDUCE":
            return op

    def _ref(in0, in1, c0, c1, c2):
        t = (in0.astype(np.float32) * c0 + c1).astype(np.float32)
        for _ in range(5):
            t = (t * t).astype(np.float32)
        return t, t.reshape(t.shape[0], -1).sum(axis=-1, keepdims=True)

    body = Src0 * C0 + C1
    for _ in range(5):
        body = sq(body)
    op = dvo.DveOp(
        "EXP_SQ_REDUCE",
        Spec(body=body, accum=_add, accum_init=Zero, reference=_ref),
        subdim=False,
        uops_sha={"v3": "ea86ec6fb1475bcb"},
    )
    dvo._SUB_OPCODE_FOR_NAME["EXP_SQ_REDUCE"] = (
        max(dvo._SUB_OPCODE_FOR_NAME.values()) + 1)
    dvo.OPS.append(op)
    return op


def _build_program():
    import concourse.tile as tile
    from concourse import bacc, bass, mybir

    EXP_SQ = _register_exp_sq()

    f32 = mybir.dt.float32
    f16 = mybir.dt.float16
    fp8 = mybir.dt.float8e4
    i32 = mybir.dt.int32
    Alu = mybir.AluOpType
    Act = mybir.ActivationFunctionType

    nc = bacc.Bacc("TRN2", target_bir_lowering=False, debug=False,
                   num_devices=N_CORES)

    l8 = nc.dram_tensor("l8", [EXT, V], fp8, kind="ExternalInput").ap()
    ids = nc.dram_tensor("ids_loc", [EXT, 1], i32, kind="ExternalInput").ap()
    maskd = nc.dram_tensor("mask_loc", [EXT, 1], f32, kind="ExternalInput").ap()
    hidt = nc.dram_tensor("hidden_t", [H, EXT], f16, kind="ExternalInput").ap()
    wt = nc.dram_tensor("wt", [H, K * OC], f16, kind="ExternalInput").ap()
    wsurp = nc.dram_tensor("wsurp", [K, OC], f16, kind="ExternalInput").ap()
    convb = nc.dram_tensor("convb", [OC, 1], f32, kind="ExternalInput").ap()
    fcw = nc.dram_tensor("fcw", [OC, 3 * PO_LOC], f32, kind="ExternalInput").ap()
    sentv = nc.dram_tensor("sentv", [128, 1], f32, kind="ExternalInput").ap()
    sentw = nc.dram_tensor("sentw", [128, 3], f32, kind="ExternalInput").ap()
    fcb = nc.dram_tensor("fcb", [3, 1], f32, kind="ExternalInput").ap()
    out = nc.dram_tensor("out_loc", [3, 1], f32, kind="ExternalOutput").ap()

    l8_flat = bass.AP(l8.tensor, 0, [[1, EXT * V], [1, 1]])

    NT = 4                             # main row tiles of 128
    NHALO = EXT - 512                  # 2 halo rows, packed [128, HF]
    HQ = 128 // NHALO                  # partitions per halo row
    HF = V // HQ                       # free elems per partition

    with tile.TileContext(nc) as tc:
        with (
            tc.tile_pool(name="lps", bufs=2) as lps,        # scalar fp8 chunks
            tc.tile_pool(name="lpd", bufs=2) as lpd,        # dve fp8 chunks
            tc.tile_pool(name="scs", bufs=1) as scs,        # scalar exp scratch
            tc.tile_pool(name="scd", bufs=1) as scd,        # dve exp scratch
            tc.tile_pool(name="big", bufs=1) as big,        # resident
            tc.tile_pool(name="sm", bufs=12) as sm,         # small stats
            tc.tile_pool(name="ps_y", bufs=1, space="PSUM") as ps_y,
            tc.tile_pool(name="ps_h", bufs=1, space="PSUM") as ps_h,
        ):
            # ---- first-wave chunk DMAs (2-deep per engine, tapered) ----
            sums_s = big.tile([128, 16], f32, tag="sums_s")
            sums_d = big.tile([128, 16], f32, tag="sums_d")
            xs_tiles = []
            for k in range(2):
                t, c0, c1 = SCH[k]
                x_sb = lps.tile([128, VS], fp8, tag="xs")
                nc.scalar.dma_start(out=x_sb[:, :c1 - c0],
                                    in_=l8[128 * t:128 * t + 128, c0:c1])
                xs_tiles.append(x_sb)
            xd_tiles = []
            for k in range(2):
                t, c0, c1 = DCH[k]
                x_sb = lpd.tile([128, DW], fp8, tag="xd")
                nc.sync.dma_start(out=x_sb[:, :c1 - c0],
                                  in_=l8[128 * t:128 * t + 128, c0:c1])
                xd_tiles.append(x_sb)

            # ---- halo rows (2): vocab packed across partitions ----
            hx = sm.tile([128, HF], fp8, tag="hx")
            halo_src = bass.AP(l8.tensor, 512 * V,
                               [[V, NHALO], [HF, HQ], [1, HF]])
            nc.sync.dma_start(out=hx[:], in_=halo_src)
            hscr = sm.tile([128, HF], f16, tag="hscr")
            hsums = sm.tile([128, 1], f32, tag="hsums")
            nc.scalar.activation(out=hscr[:], in_=hx[:], func=Act.Exp,
                                 accum_out=hsums[:])

            # ---- gpsimd setup: memsets, index math (no queue traffic) ----
            nc.gpsimd.memset(sums_s[:], 0.0)
            nc.gpsimd.memset(sums_d[:], 0.0)
            ids_all = sm.tile([128, NT], i32, tag="ids")
            nc.gpsimd.dma_start(out=ids_all[:],
                                in_=bass.AP(ids.tensor, 0, [[1, 128], [128, NT]]))
            m_all = big.tile([128, NT + 1], f32, tag="m")
            nc.gpsimd.dma_start(out=m_all[:, :NT],
                                in_=bass.AP(maskd.tensor, 0, [[1, 128], [128, NT]]))
            nc.gpsimd.memset(m_all[:, NT:], 0.0)
            nc.gpsimd.dma_start(out=m_all[:NHALO, NT:], in_=maskd[512:EXT, :])
            se_all = big.tile([128, NT + 1], f32, tag="se")
            nc.gpsimd.memset(se_all[:, NT:], 1.0)   # ln(1)=0 on unused lanes
            hsel = big.tile([128, NHALO], f32, tag="hsel")
            nc.gpsimd.memset(hsel[:], 0.0)
            for a in range(NHALO):
                nc.gpsimd.memset(hsel[a * HQ:(a + 1) * HQ, a:a + 1], 1.0)
            ones_sb = big.tile([128, 1], f32, tag="ones")
            nc.gpsimd.memset(ones_sb[:], 1.0)

            iota_t = sm.tile([128, NT], i32, tag="iota")
            nc.gpsimd.iota(iota_t[:], pattern=[[1, NT]], base=0,
                           channel_multiplier=0)
            nc.gpsimd.tensor_scalar(out=iota_t[:], in0=iota_t[:],
                                    scalar1=128 * V, scalar2=None, op0=Alu.mult)
            iota_p = sm.tile([128, 1], i32, tag="iotap")
            nc.gpsimd.iota(iota_p[:], pattern=[[1, 1]], base=0,
                           channel_multiplier=V)
            flat_all = sm.tile([128, NT], i32, tag="flat")
            nc.gpsimd.tensor_tensor(out=flat_all[:], in0=ids_all[:],
                                    in1=iota_t[:], op=Alu.add)
            nc.gpsimd.tensor_tensor(out=flat_all[:], in0=flat_all[:],
                                    in1=iota_p[:].to_broadcast([128, NT]),
                                    op=Alu.add)
            hrb = sm.tile([128, 1], i32, tag="hrb")
            nc.gpsimd.iota(hrb[:NHALO, :], pattern=[[1, 1]], base=512 * V,
                           channel_multiplier=V)
            hids = sm.tile([128, 1], i32, tag="hids")
            nc.gpsimd.dma_start(out=hids[:NHALO, :], in_=ids[512:EXT, :])
            hfl = sm.tile([128, 1], i32, tag="hfl")
            nc.gpsimd.tensor_tensor(out=hfl[:NHALO, :], in0=hids[:NHALO, :],
                                    in1=hrb[:NHALO, :], op=Alu.add)
            g8 = sm.tile([128, NT + 1], fp8, tag="g8")
            nc.gpsimd.memset(g8[:], 0.0)

            # halo partition-reduce via select matmul (PE)
            psum_h = ps_h.tile([NHALO, 1], f32, tag="ph")
            nc.tensor.matmul(out=psum_h[:], lhsT=hsel[:], rhs=hsums[:],
                             start=True, stop=True)

            # small resident weights (sync ring; tiny)
            wsurp_sb = big.tile([K, OC], f16, tag="wsurp")
            nc.sync.dma_start(out=wsurp_sb[:], in_=wsurp)
            convb_sb = big.tile([OC, 1], f32, tag="convb")
            nc.sync.dma_start(out=convb_sb[:], in_=convb)
            fcw_sb = big.tile([OC, 3 * PO_LOC], f32, tag="fcw")
            nc.sync.dma_start(out=fcw_sb[:], in_=fcw)
            sentv_sb = big.tile([128, 1], f32, tag="sentv")
            nc.sync.dma_start(out=sentv_sb[:], in_=sentv)
            sentw_sb = big.tile([128, 3], f32, tag="sentw")
            nc.sync.dma_start(out=sentw_sb[:], in_=sentw)
            fcb_sb = big.tile([3, 1], f32, tag="fcb")
            nc.sync.dma_start(out=fcb_sb[:], in_=fcb)

            # ---- main stream: per-engine DMA rings + exp pipelines ----
            wtile = big.tile([128, 16 * K * OC], f16, tag="wtile")
            xt = big.tile([128, 16 * EXT], f16, tag="xt")
            es = scs.tile([128, VS], f16, tag="es")
            ed = scd.tile([128, DW], f16, tag="ed")
            for k in range(len(SCH)):
                t, c0, c1 = SCH[k]
                nc.scalar.activation(
                    out=es[:, :c1 - c0], in_=xs_tiles[k][:, :c1 - c0],
                    func=Act.Exp, accum_out=sums_s[:, SCOL[k]:SCOL[k] + 1])
                if k == 3:
                    # big resident loads: issued ~1 tile in, land mid-stream,
                    # well before the conv matmuls need them
                    nc.scalar.dma_start(
                        out=wtile[:].rearrange("p (c v) -> p c v", c=16),
                        in_=wt.rearrange("(c p) v -> p c v", p=128))
                    nc.scalar.dma_start(
                        out=xt[:].rearrange("p (c v) -> p c v", c=16),
                        in_=hidt.rearrange("(c p) v -> p c v", p=128))
                if k + 2 < len(SCH):
                    t2, d0, d1 = SCH[k + 2]
                    x_sb = lps.tile([128, VS], fp8, tag="xs")
                    nc.scalar.dma_start(out=x_sb[:, :d1 - d0],
                                        in_=l8[128 * t2:128 * t2 + 128, d0:d1])
                    xs_tiles.append(x_sb)
                t, c0, c1 = DCH[k]
                nc.vector._custom_dve(
                    EXP_SQ, out=ed[:, :c1 - c0], in0=xd_tiles[k][:, :c1 - c0],
                    s0=C0V, s1=1.0, accum_out=sums_d[:, SCOL[k]:SCOL[k] + 1])
                if k + 2 < len(DCH):
                    t2, d0, d1 = DCH[k + 2]
                    x_sb = lpd.tile([128, DW], fp8, tag="xd")
                    nc.sync.dma_start(out=x_sb[:, :d1 - d0],
                                      in_=l8[128 * t2:128 * t2 + 128, d0:d1])
                    xd_tiles.append(x_sb)

            # ---- token-logit gathers: delayed behind a mid-stream chunk so
            # their scattered DRAM reads don't crowd the queues at startup
            dscr = sm.tile([1, 1], f32, tag="dscr")
            nc.gpsimd.tensor_scalar(out=dscr[:], in0=xd_tiles[4][0:1, 0:1],
                                    scalar1=1.0, scalar2=None, op0=Alu.mult)
            for t in range(NT):
                # HW DGE honors only one index per partition per transfer
                nc.gpsimd.indirect_dma_start(
                    out=g8[:, t:t + 1], out_offset=None, in_=l8_flat,
                    in_offset=bass.IndirectOffsetOnAxis(
                        ap=flat_all[:, t:t + 1], axis=0))
            nc.gpsimd.indirect_dma_start(
                out=g8[:NHALO, NT:], out_offset=None, in_=l8_flat,
                in_offset=bass.IndirectOffsetOnAxis(ap=hfl[:NHALO, :1], axis=0))

            # ---- conv: 80 hidden matmuls accumulate into one PSUM bank ----
            psum_y = ps_y.tile([OC, Y_LOC], f32, tag="y")
            first = True
            for cc in range(16):
                for k in range(K):
                    nc.tensor.matmul(
                        out=psum_y[:],
                        lhsT=wtile[:, cc * 640 + k * 128: cc * 640 + (k + 1) * 128],
                        rhs=xt[:, cc * EXT + k: cc * EXT + k + Y_LOC],
                        start=first,
                        stop=False,
                    )
                    first = False

            # ---- batched LSE -> surp -> srow ----
            nc.vector.tensor_copy(out=se_all[:NHALO, NT:], in_=psum_h[:])
            g_all = big.tile([128, NT + 1], f32, tag="g")
            nc.vector.tensor_copy(out=g_all[:], in_=g8[:])
            ssr = sm.tile([128, NT], f32, tag="ssr")
            nc.vector.tensor_reduce(
                out=ssr[:], in_=sums_s[:].rearrange("p (t j) -> p t j", t=NT),
                axis=mybir.AxisListType.X, op=Alu.add)
            sdr = sm.tile([128, NT], f32, tag="sdr")
            nc.vector.tensor_reduce(
                out=sdr[:], in_=sums_d[:].rearrange("p (t j) -> p t j", t=NT),
                axis=mybir.AxisListType.X, op=Alu.add)
            nc.vector.tensor_tensor(out=se_all[:, 0:NT], in0=ssr[:],
                                    in1=sdr[:], op=Alu.add)

            lse_all = sm.tile([128, NT + 1], f32, tag="lse")
            nc.scalar.activation(out=lse_all[:], in_=se_all[:], func=Act.Ln)
            surp_all = sm.tile([128, NT + 1], f16, tag="surp")
            nc.vector.tensor_tensor(out=surp_all[:], in0=lse_all[:],
                                    in1=g_all[:], op=Alu.subtract)
            nc.vector.tensor_scalar(out=surp_all[:], in0=surp_all[:],
                                    scalar1=COMP, op0=Alu.subtract,
                                    scalar2=LOG2E, op1=Alu.mult)
            nc.vector.tensor_tensor(out=surp_all[:], in0=surp_all[:],
                                    in1=m_all[:], op=Alu.mult)
            # srow (f16, in-flight cast), spread across DGE rings
            srow = big.tile([1, EXT], f16, tag="srow")
            nc.gpsimd.dma_start(out=srow[0:1, 0:128], in_=surp_all[:, 0:1])
            nc.gpsimd.dma_start(out=srow[0:1, 512:EXT],
                                in_=surp_all[:NHALO, NT:])
            nc.scalar.dma_start(out=srow[0:1, 128:256], in_=surp_all[:, 1:2])
            nc.scalar.dma_start(out=srow[0:1, 256:384], in_=surp_all[:, 2:3])
            nc.sync.dma_start(out=srow[0:1, 384:512], in_=surp_all[:, 3:4])

            # ---- surp channel: one contract-5 matmul closes the accumulation
            s5 = big.tile([K, Y_LOC], f16, tag="s5")
            s5_src = bass.AP(srow[:].tensor, srow[:].offset,
                             [[1, 1], [1, K], [1, Y_LOC]])
            nc.gpsimd.dma_start(out=s5[:], in_=s5_src)
            nc.tensor.matmul(
                out=psum_y[:],
                lhsT=wsurp_sb[:],
                rhs=s5[:],
                start=False,
                stop=True,
            )

            # ---- maxpool(5) + bias + relu: two fused ops ----
            pooled = big.tile([OC, PO_LOC], f32, tag="pooled")
            nc.vector.tensor_reduce(
                out=pooled[:],
                in_=psum_y[:].rearrange("p (a b) -> p a b", b=K),
                axis=mybir.AxisListType.X, op=Alu.max)
            nc.vector.tensor_scalar(out=pooled[:], in0=pooled[:],
                                    scalar1=convb_sb[:, 0:1], op0=Alu.add,
                                    scalar2=0.0, op1=Alu.max)

            # ---- FC partial: red[oc, l] = sum_p pooled*fcw ----
            red = big.tile([OC, 3], f32, tag="red")
            fc_scr = big.tile([OC, PO_LOC], f32, tag="fcscr")
            for l in range(3):
                nc.vector.tensor_tensor(
                    out=fc_scr[:],
                    in0=pooled[:],
                    in1=fcw_sb[:, l * PO_LOC:(l + 1) * PO_LOC],
                    op=Alu.mult)
                nc.vector.tensor_reduce(
                    out=red[:, l:l + 1], in_=fc_scr[:],
                    axis=mybir.AxisListType.X, op=Alu.add)
            # sentiment branch (zeroed on h==1 cores)
            rs = sm.tile([128, 1], f32, tag="rs")
            nc.vector.tensor_scalar(out=rs[:], in0=sentv_sb[:], scalar1=0.0,
                                    scalar2=None, op0=Alu.max)
            tmp3 = sm.tile([128, 3], f32, tag="tmp3")
            nc.vector.tensor_scalar(out=tmp3[:], in0=sentw_sb[:],
                                    scalar1=rs[:, 0:1], scalar2=None,
                                    op0=Alu.mult)
            nc.vector.tensor_tensor(out=red[:], in0=red[:], in1=tmp3[:],
                                    op=Alu.add)

            psum_out = ps_h.tile([3, 1], f32, tag="po")
            nc.tensor.matmul(out=psum_out[:], lhsT=red[:], rhs=ones_sb[:],
                             start=True, stop=True)
            out_sb = sm.tile([3, 1], f32, tag="outsb")
            nc.vector.tensor_tensor(out=out_sb[:], in0=psum_out[:],
                                    in1=fcb_sb[:], op=Alu.add)
            nc.sync.dma_start(out=out, in_=out_sb[:])

    nc.compile()
    return nc


def _prep_core_inputs(core, l8_full, input_ids, attention_mask, sentiment,
                      hidden, conv_w, conv_b, fc_w, fc_b):
    b, h = core // 2, core % 2
    g0 = Y_LOC * h
    ext0 = g0 - 2

    lg = np.zeros((EXT, V), NP8)
    idl = np.zeros((EXT, 1), np.int32)
    mk = np.zeros((EXT, 1), np.float32)
    hdt = np.zeros((H, EXT), np.float16)
    lo = max(0, -ext0)            # local index where valid rows start
    s0, s1 = ext0 + lo, ext0 + EXT
    lg[lo:] = l8_full[b, s0:s1]
    idl[lo:, 0] = input_ids[b, s0:s1].astype(np.int32)
    mk[lo:, 0] = attention_mask[b, s0:s1]
    hdt[:, lo:] = hidden[b, s0:s1].T.astype(np.float16)

    wt = np.ascontiguousarray(
        conv_w[:, :H, :].transpose(1, 2, 0).reshape(H, K * OC)).astype(np.float16)
    ws = np.ascontiguousarray(conv_w[:, H, :].T).astype(np.float16)  # [K, OC]
    cb = np.ascontiguousarray(conv_b[:, None])             # [OC, 1]

    w3 = fc_w[:, :OC * 204].reshape(3, OC, 204)
    fcw = np.ascontiguousarray(
        w3[:, :, h * PO_LOC:(h + 1) * PO_LOC].transpose(1, 0, 2).reshape(OC, 3 * PO_LOC))

    sv = np.zeros((128, 1), np.float32)
    sw = np.zeros((128, 3), np.float32)
    fb = np.zeros((3, 1), np.float32)
    if h == 0:
        sv[:3, 0] = sentiment[b]
        sw[:3, :] = fc_w[:, OC * 204:].T                   # [3 j, 3 l]
        fb[:, 0] = fc_b

    return {
        "l8": lg, "ids_loc": idl, "mask_loc": mk, "hidden_t": hdt,
        "wt": wt, "wsurp": ws, "convb": cb, "fcw": fcw,
        "sentv": sv, "sentw": sw, "fcb": fb,
    }


def _install_ntff_hook():
    import sys
    import types
    try:
        import antenv
        from trn_agent_boot.trn_boot import _ntff_profile_via_ctypes
    except ImportError:
        return
    if "antenv.axon_hooks" in sys.modules:
        return
    mod = types.ModuleType("antenv.axon_hooks")
    _h = [None]
    mod.set_axon_ntff_profile_hook = lambda hk: _h.__setitem__(0, hk)
    mod.get_axon_ntff_profile_hook = lambda: _h[0]
    sys.modules["antenv.axon_hooks"] = mod
    antenv.axon_hooks = mod
    try:
        mod.set_axon_ntff_profile_hook(
            _ntff_profile_via_ctypes('/opt/axon/libaxon_pjrt.so'))
    except Exception:
        pass


def kernel(input_ids, attention_mask, sentiment, logits, hidden,
           conv_w, conv_b, fc_w, fc_b, _trace=False):
    from concourse.bass_utils import run_bass_kernel_spmd

    input_ids = np.asarray(input_ids)
    attention_mask = np.asarray(attention_mask, np.float32)
    sentiment = np.asarray(sentiment, np.float32)
    logits = np.asarray(logits, np.float32)
    hidden = np.asarray(hidden, np.float32)
    conv_w = np.asarray(conv_w, np.float32)
    conv_b = np.asarray(conv_b, np.float32)
    fc_w = np.asarray(fc_w, np.float32)
    fc_b = np.asarray(fc_b, np.float32)

    if "nc" not in _CACHE:
        _CACHE["nc"] = _build_program()
    nc = _CACHE["nc"]

    l8_full = logits.astype(NP8)       # one cast, sliced per core below
    in_maps = [
        _prep_core_inputs(c, l8_full, input_ids, attention_mask, sentiment,
                          hidden, conv_w, conv_b, fc_w, fc_b)
        for c in range(N_CORES)
    ]
    if _trace:
        _install_ntff_hook()
    res = run_bass_kernel_spmd(nc, in_maps, list(range(N_CORES)), trace=_trace)
    _CACHE["last_result"] = res

    out = np.zeros((B, 3), np.float32)
    for b in range(B):
        out[b] = (res.results[2 * b]["out_loc"][:, 0]
                  + res.results[2 * b + 1]["out_loc"][:, 0])
    return out
